# revision 3
# baseline (speedup 1.0000x reference)
"""Trainium2 Bass kernel v3 for the water-network leak MSE model.

v2 -> v3: interleaved emission (PE never waits a whole phase), 2-bank q PSUM
tiles with ACT Abs drains, Mitchell affine straight off the abs bits (no mask
pass), hl8 TT reads PSUM f32 (sign source, one less drain), relu moved to DVE
tensor_scalar (H weights carry -2), d_leak injected into the residual PSUM by
the PE so ACT Square+accum reads PSUM directly (no DVE rfin pass).

Scales: D*128 fp8, q-weights invev*k1/128, PM table /16 with 16I injects,
hl8 = hL/8192 (Mitchell const (t+870.6)*0.852), H weights -2*invp,
rl = max(hp + hsup/4096, 0) = Hp/4096, sq = sqrt(Hp)/64, M table *c0*8192,
AM table *8 with 16I injects, res weights (A0inv - I), residual = 128*r,
stats accumulate (128r)^2; host divides by 128^2*S*N.
"""

import math

import numpy as np
import ml_dtypes

P = 128
N_CORES = 8
S_TOTAL = 16384
SC = S_TOTAL // N_CORES
CH = 512
NCH = SC // CH
N_NODES = 512
N_PIPES = 1024
N_DEM = 256
G_ACC = 9.80665

BF16 = ml_dtypes.bfloat16
E4 = ml_dtypes.float8_e4m3

_MODULE_CACHE: dict = {}


def _build_module():
    import concourse.bacc as bacc
    import concourse.mybir as mybir
    import concourse.tile as tile

    f32 = mybir.dt.float32
    bf16 = mybir.dt.bfloat16
    f8 = mybir.dt.float8e4
    i16 = mybir.dt.int16
    AF = mybir.ActivationFunctionType
    OP = mybir.AluOpType
    DR = mybir.MatmulPerfMode.DoubleRow

    nc = bacc.Bacc(trn_type="TRN2", target_bir_lowering=False, debug=False)

    import types as _types
    from concourse.hw_specs import get_activation_tables as _gat
    import bass_rust as _bass_rust

    _OURS = {AF.Abs, AF.Relu, AF.Square, AF.Sqrt, AF.Identity, AF.Copy,
             AF.Sign, AF.MemsetZero}

    def _patched_act_table_loads(self):
        has_activation = any(
            isinstance(i, mybir.InstActivation)
            for b in self.main_func.blocks
            for i in b.instructions
        )
        if not has_activation:
            return
        tables = []
        for name, fns in _gat(self.m.arch).items():
            if name != "sqrt_and_others":
                fns = fns - _OURS
            tables.append((name, fns))
        _bass_rust.insert_act_table_loads(self, tables)

    nc.insert_act_table_loads = _types.MethodType(_patched_act_table_loads, nc)

    dt8_d = nc.dram_tensor("dt8", [P, 2, SC], f8, kind="ExternalInput").ap()
    invev_d = nc.dram_tensor("invev8", [P, 2, N_PIPES], f8, kind="ExternalInput").ap()
    a0inv_d = nc.dram_tensor("a0inv8", [P, 2, N_NODES], f8, kind="ExternalInput").ap()
    invpt_d = nc.dram_tensor("invpt8", [P, 8, N_NODES], f8, kind="ExternalInput").ap()
    ident_d = nc.dram_tensor("ident16", [P, P], f8, kind="ExternalInput").ap()
    nident_d = nc.dram_tensor("nident", [P, P], bf16, kind="ExternalInput").ap()
    hsup_d = nc.dram_tensor("hsup4", [P, 4], f32, kind="ExternalInput").ap()
    g8_ds = [
        nc.dram_tensor(f"g8_{c}", [P, 12, CH], f8, kind="ExternalInput").ap()
        for c in range(NCH)
    ]
    gm_ds = [
        nc.dram_tensor(f"gm_{c}", [P, 4, CH], bf16, kind="ExternalInput").ap()
        for c in range(NCH)
    ]
    out_d = nc.dram_tensor("out_stats", [P, 4 * NCH], f32, kind="ExternalOutput").ap()

    with tile.TileContext(nc) as tc:
        with (
            tc.tile_pool(name="const", bufs=1) as cpool,
            tc.tile_pool(name="gat", bufs=1) as gpool,
            tc.tile_pool(name="work", bufs=1) as wpool,
            tc.tile_pool(name="qps", bufs=2, space="PSUM") as qpool,
            tc.tile_pool(name="hps", bufs=2, space="PSUM") as hpool,
            tc.tile_pool(name="rps", bufs=2, space="PSUM") as rpool,
        ):
            invev8 = cpool.tile_from(invev_d)
            dt8 = cpool.tile_from(dt8_d)
            ident16 = cpool.tile_from(ident_d)

            g8s, gms = [], []
            for sc in range(NCH):
                g8 = gpool.tile([P, 12, CH], f8, tag="g8", bufs=3, name=f"g8t{sc}")
                nc.sync.dma_start(g8, g8_ds[sc])
                gm = gpool.tile([P, 4, CH], bf16, tag="gm", bufs=3, name=f"gmt{sc}")
                nc.sync.dma_start(gm, gm_ds[sc])
                g8s.append(g8)
                gms.append(gm)
                if sc == 0:
                    invpt8 = cpool.tile_from(invpt_d)
                    a0inv8 = cpool.tile_from(a0inv_d)
                    nident = cpool.tile_from(nident_d)
                    hsup4 = cpool.tile_from(hsup_d)
                    stats = cpool.tile([P, 4 * NCH], f32, tag="stats")

            # per-chunk state tiles
            state = {}

            def q_half(sc, half):
                """q tiles for pipe-block pairs (2 per half): matmuls + abs
                drain + mitchell affine + hl8 fp8 conversion."""
                s0 = sc * CH
                if half == 0:
                    state[sc] = {
                        "qabs": wpool.tile(
                            [P, 8, CH], bf16, tag="qabs", bufs=2,
                            name=f"qabs{sc}"),
                        "e852": wpool.tile(
                            [P, 8, CH], i16, tag="e852", bufs=2,
                            name=f"e852{sc}"),
                        "hl8": wpool.tile(
                            [P, 8, CH], f8, tag="hl8", bufs=2,
                            name=f"hl8{sc}"),
                        "rl": wpool.tile(
                            [P, 4, CH], bf16, tag="rl", bufs=2,
                            name=f"rl{sc}"),
                        "sq": wpool.tile(
                            [P, 4, CH], bf16, tag="sq", bufs=2,
                            name=f"sq{sc}"),
                        "dl": wpool.tile(
                            [P, 4, CH], bf16, tag="dl", bufs=2,
                            name=f"dl{sc}"),
                    }
                st = state[sc]
                for h in (2 * half, 2 * half + 1):
                    qp2 = qpool.tile([P, 2, CH], f32, tag="qp", name=f"qp{sc}_{h}")
                    for j in range(2):
                        pc = 2 * h + j
                        nc.tensor.matmul(
                            qp2[:, j, :],
                            invev8[:, :, pc * P : (pc + 1) * P],
                            dt8[:, :, s0 : s0 + CH],
                            start=True, stop=False, perf_mode=DR,
                        )
                        nc.tensor.matmul(
                            qp2[:, j, :], ident16, g8s[sc][:, pc, :],
                            start=False, stop=True,
                        )
                    sl = st["qabs"][:, 2 * h : 2 * h + 2, :]
                    nc.scalar.activation(sl, qp2, AF.Abs)
                    esl = st["e852"][:, 2 * h : 2 * h + 2, :]
                    nc.vector.tensor_scalar(
                        esl, sl.bitcast(i16), 870.6, 0.852, OP.add, OP.mult,
                    )
                    nc.vector.tensor_tensor(
                        st["hl8"][:, 2 * h : 2 * h + 2, :],
                        qp2, esl.bitcast(bf16), OP.mult,
                    )

            def h_half(sc, half):
                """H for node-blocks (2 per half): DR matmuls + DVE relu +
                ACT sqrt + Pool dl."""
                st = state[sc]
                for n in (2 * half, 2 * half + 1):
                    hp = hpool.tile([P, CH], f32, tag="hp", name=f"hp{sc}_{n}")
                    for kp in range(4):
                        nc.tensor.matmul(
                            hp,
                            invpt8[:, 2 * kp : 2 * kp + 2, n * P : (n + 1) * P],
                            st["hl8"][:, 2 * kp : 2 * kp + 2, :],
                            start=(kp == 0), stop=(kp == 3), perf_mode=DR,
                        )
                    nc.vector.tensor_scalar(
                        st["rl"][:, n, :], hp, hsup4[:, n : n + 1], 0.0,
                        OP.add, OP.max,
                    )
                eng = nc.gpsimd if sc < NCH - 1 else nc.vector
                for n in (2 * half, 2 * half + 1):
                    nc.scalar.activation(
                        st["sq"][:, n, :], st["rl"][:, n, :], AF.Sqrt,
                    )
                    eng.tensor_tensor(
                        st["dl"][:, n, :], gms[sc][:, n, :],
                        st["sq"][:, n, :], OP.mult,
                    )

            def r_half(sc, half):
                """residual for node-blocks (2 per half): DR + AM inject +
                (-I)*dl inject, ACT Square+accum from PSUM."""
                s0 = sc * CH
                st = state[sc]
                scr = wpool.tile([P, 2, CH], bf16, tag="scr", bufs=2,
                                 name=f"scr{sc}_{half}")
                rps = []
                for n in (2 * half, 2 * half + 1):
                    rp = rpool.tile([P, CH], f32, tag="rp", name=f"rp{sc}_{n}")
                    nc.tensor.matmul(
                        rp,
                        a0inv8[:, :, n * P : (n + 1) * P],
                        dt8[:, :, s0 : s0 + CH],
                        start=True, stop=False, perf_mode=DR,
                    )
                    nc.tensor.matmul(
                        rp, ident16, g8s[sc][:, 8 + n, :],
                        start=False, stop=False,
                    )
                    rps.append(rp)
                for j, n in enumerate((2 * half, 2 * half + 1)):
                    nc.tensor.matmul(
                        rps[j], nident, st["dl"][:, n, :],
                        start=False, stop=True,
                    )
                    col = 4 * sc + 2 * half + j
                    nc.scalar.activation(
                        scr[:, j, :], rps[j], AF.Square,
                        accum_out=stats[:, col : col + 1],
                    )

            # interleaved emission: PE queue alternates q / H / res work so
            # it never drains a whole phase while DVE/ACT catch up
            q_half(0, 0)
            q_half(0, 1)
            q_half(1, 0)
            q_half(1, 1)
            for sc in range(NCH):
                h_half(sc, 0)
                if sc + 2 < NCH:
                    q_half(sc + 2, 0)
                h_half(sc, 1)
                if sc + 2 < NCH:
                    q_half(sc + 2, 1)
                r_half(sc, 0)
                r_half(sc, 1)
            nc.sync.dma_start(out_d, stats)

    nc.compile()
    return nc


def _host_prep(inputs):
    D = np.ascontiguousarray(np.asarray(inputs["D"], np.float32))
    leak = np.asarray(inputs["leak_id"]).reshape(-1).astype(np.int64)
    A0 = np.asarray(inputs["A0"], np.float32).astype(np.float64)
    inv = np.asarray(inputs["inv"], np.float32).astype(np.float64)
    M = np.asarray(inputs["M"], np.float32).astype(np.float64)
    supply = np.asarray(inputs["supply"], np.float32).astype(np.float64)
    L = np.asarray(inputs["L"], np.float32).astype(np.float64)
    d = np.asarray(inputs["d"], np.float32).astype(np.float64)
    C = np.asarray(inputs["C"], np.float32).astype(np.float64)
    a = float(np.asarray(inputs["a"]))
    Cd = float(np.asarray(inputs["Cd"]))
    W1 = np.asarray(inputs["W1"], np.float64)
    b1 = np.asarray(inputs["b1"], np.float64)
    W2 = np.asarray(inputs["W2"], np.float64)
    b2 = np.asarray(inputs["b2"], np.float64)
    W3 = np.asarray(inputs["W3"], np.float64)
    b3 = np.asarray(inputs["b3"], np.float64)
    base = np.asarray(inputs["base"], np.float64)

    ids = np.arange(N_PIPES, dtype=np.float64)[:, None]
    h = np.tanh(ids @ W1 + b1)
    h = np.tanh(h @ W2 + b2)
    table = base + (h @ W3 + b3)[:, 0]

    perm = np.concatenate([np.arange(0, N_NODES, 2), np.arange(1, N_NODES, 2)])
    invp = inv[perm]
    A0p = A0[perm]
    Mp = M[perm]
    inv_ev = invp[:N_DEM]

    K = 10.667 * C**-1.852 * d**-4.871 * L
    k1 = K ** (1.0 / 1.852)
    c0 = Cd * a * math.sqrt(2.0 * G_ACC)

    PM = inv.T @ M
    PMn = (PM * table[None, :]) * k1[:, None]
    AMn = (A0p @ PM) * table[None, :]
    A0inv = A0p @ inv_ev.T
    Ipad = np.zeros((N_NODES, N_DEM))
    Ipad[np.arange(N_DEM), np.arange(N_DEM)] = 1.0

    pm8_t = (PMn / 16.0).T.astype(np.float32).astype(E4)
    am8_t = (AMn * 8.0).T.astype(np.float32).astype(E4)
    m16_t = (Mp * (c0 * 8192.0)).T.astype(np.float32).astype(BF16)

    def blocks3(mat, nb):
        rows, cols = mat.shape
        assert rows == nb * P
        return np.ascontiguousarray(
            np.transpose(mat.reshape(nb, P, cols), (1, 0, 2))
        )

    invev8 = blocks3(
        (inv_ev * k1[None, :] / 128.0).astype(np.float32).astype(E4), 2)
    a0inv8 = blocks3(
        np.ascontiguousarray((A0inv - Ipad).T).astype(np.float32).astype(E4), 2)
    invpt8 = blocks3(
        np.ascontiguousarray((-2.0 * invp).T).astype(np.float32).astype(E4), 8)
    ident16 = (np.eye(P) * 16.0).astype(np.float32).astype(E4)
    nident = (-np.eye(P)).astype(np.float32).astype(BF16)
    hsup4 = np.ascontiguousarray(
        ((invp @ supply) / 4096.0).reshape(4, P).T).astype(np.float32)

    per_core = []
    for c in range(N_CORES):
        Dc = D[c * SC : (c + 1) * SC]
        DT = (Dc.T * 128.0).astype(np.float32).astype(E4)
        dt8 = blocks3(DT, 2)
        lc = leak[c * SC : (c + 1) * SC]
        g8s, gms = [], []
        for sc in range(NCH):
            li = lc[sc * CH : (sc + 1) * CH]
            gpm = blocks3(np.ascontiguousarray(pm8_t[li].T), 8)
            gam = blocks3(np.ascontiguousarray(am8_t[li].T), 4)
            g8s.append(np.ascontiguousarray(
                np.concatenate([gpm, gam], axis=1)))
            gms.append(blocks3(np.ascontiguousarray(m16_t[li].T), 4))
        per_core.append((dt8, g8s, gms))

    shared = {
        "invev8": invev8,
        "a0inv8": a0inv8,
        "invpt8": invpt8,
        "ident16": ident16,
        "nident": nident,
        "hsup4": hsup4,
    }
    return shared, per_core


LAST_RESULTS = None


def kernel(**inputs) -> np.ndarray:
    global LAST_RESULTS
    from concourse.bass_utils import run_bass_kernel_spmd

    shared, per_core = _host_prep(inputs)

    if "nc" not in _MODULE_CACHE:
        _MODULE_CACHE["nc"] = _build_module()
    nc = _MODULE_CACHE["nc"]

    in_maps = []
    for c in range(N_CORES):
        dt8, g8s, gms = per_core[c]
        m = dict(shared)
        m["dt8"] = dt8
        for sc in range(NCH):
            m[f"g8_{sc}"] = g8s[sc]
            m[f"gm_{sc}"] = gms[sc]
        in_maps.append(m)

    import os

    res = run_bass_kernel_spmd(
        nc,
        in_maps,
        core_ids=list(range(N_CORES)),
        trace=bool(os.environ.get("BASS_TRACE")),
    )
    LAST_RESULTS = res

    total = 0.0
    for r in res.results:
        total += float(r["out_stats"].astype(np.float64).sum())
    return np.float32(total / (128.0 * 128.0 * S_TOTAL * N_NODES))


# revision 4
# speedup vs baseline: 1.0257x; 1.0257x over previous
"""Trainium2 Bass kernel v3 for the water-network leak MSE model.

v2 -> v3: interleaved emission (PE never waits a whole phase), 2-bank q PSUM
tiles with ACT Abs drains, Mitchell affine straight off the abs bits (no mask
pass), hl8 TT reads PSUM f32 (sign source, one less drain), relu moved to DVE
tensor_scalar (H weights carry -2), d_leak injected into the residual PSUM by
the PE so ACT Square+accum reads PSUM directly (no DVE rfin pass).

Scales: D*128 fp8, q-weights invev*k1/128, PM table /16 with 16I injects,
hl8 = hL/8192 (Mitchell const (t+870.6)*0.852), H weights -2*invp,
rl = max(hp + hsup/4096, 0) = Hp/4096, sq = sqrt(Hp)/64, M table *c0*8192,
AM table *8 with 16I injects, res weights (A0inv - I), residual = 128*r,
stats accumulate (128r)^2; host divides by 128^2*S*N.
"""

import math

import numpy as np
import ml_dtypes

P = 128
N_CORES = 8
S_TOTAL = 16384
SC = S_TOTAL // N_CORES
CH = 512
NCH = SC // CH
N_NODES = 512
N_PIPES = 1024
N_DEM = 256
G_ACC = 9.80665

BF16 = ml_dtypes.bfloat16
E4 = ml_dtypes.float8_e4m3

_MODULE_CACHE: dict = {}


def _build_module():
    import concourse.bacc as bacc
    import concourse.mybir as mybir
    import concourse.tile as tile

    f32 = mybir.dt.float32
    bf16 = mybir.dt.bfloat16
    f8 = mybir.dt.float8e4
    i16 = mybir.dt.int16
    AF = mybir.ActivationFunctionType
    OP = mybir.AluOpType
    DR = mybir.MatmulPerfMode.DoubleRow

    nc = bacc.Bacc(trn_type="TRN2", target_bir_lowering=False, debug=False)

    import types as _types
    from concourse.hw_specs import get_activation_tables as _gat
    import bass_rust as _bass_rust

    _OURS = {AF.Abs, AF.Relu, AF.Square, AF.Sqrt, AF.Identity, AF.Copy,
             AF.Sign, AF.MemsetZero}

    def _patched_act_table_loads(self):
        has_activation = any(
            isinstance(i, mybir.InstActivation)
            for b in self.main_func.blocks
            for i in b.instructions
        )
        if not has_activation:
            return
        tables = []
        for name, fns in _gat(self.m.arch).items():
            if name != "sqrt_and_others":
                fns = fns - _OURS
            tables.append((name, fns))
        _bass_rust.insert_act_table_loads(self, tables)

    nc.insert_act_table_loads = _types.MethodType(_patched_act_table_loads, nc)

    dt8_d = nc.dram_tensor("dt8", [P, 2, SC], f8, kind="ExternalInput").ap()
    invev_d = nc.dram_tensor("invev8", [P, 2, N_PIPES], f8, kind="ExternalInput").ap()
    a0inv_d = nc.dram_tensor("a0inv8", [P, 2, N_NODES], f8, kind="ExternalInput").ap()
    invpt_d = nc.dram_tensor("invpt8", [P, 8, N_NODES], f8, kind="ExternalInput").ap()
    ident_d = nc.dram_tensor("ident16", [P, P], f8, kind="ExternalInput").ap()
    nident_d = nc.dram_tensor("nident", [P, P], bf16, kind="ExternalInput").ap()
    hsup_d = nc.dram_tensor("hsup4", [P, 4], f32, kind="ExternalInput").ap()
    g8_ds = [
        nc.dram_tensor(f"g8_{c}", [P, 12, CH], f8, kind="ExternalInput").ap()
        for c in range(NCH)
    ]
    gm_ds = [
        nc.dram_tensor(f"gm_{c}", [P, 4, CH], bf16, kind="ExternalInput").ap()
        for c in range(NCH)
    ]
    out_d = nc.dram_tensor("out_stats", [P, 4 * NCH], f32, kind="ExternalOutput").ap()

    with tile.TileContext(nc) as tc:
        with (
            tc.tile_pool(name="const", bufs=1) as cpool,
            tc.tile_pool(name="gat", bufs=1) as gpool,
            tc.tile_pool(name="work", bufs=1) as wpool,
            tc.tile_pool(name="qps", bufs=2, space="PSUM") as qpool,
            tc.tile_pool(name="hps", bufs=2, space="PSUM") as hpool,
            tc.tile_pool(name="rps", bufs=2, space="PSUM") as rpool,
        ):
            invev8 = cpool.tile_from(invev_d)
            dt8 = cpool.tile_from(dt8_d)
            ident16 = cpool.tile_from(ident_d)

            g8s, gms = [], []
            for sc in range(NCH):
                g8 = gpool.tile([P, 12, CH], f8, tag="g8", bufs=3, name=f"g8t{sc}")
                nc.sync.dma_start(g8, g8_ds[sc])
                gm = gpool.tile([P, 4, CH], bf16, tag="gm", bufs=3, name=f"gmt{sc}")
                nc.sync.dma_start(gm, gm_ds[sc])
                g8s.append(g8)
                gms.append(gm)
                if sc == 0:
                    invpt8 = cpool.tile_from(invpt_d)
                    a0inv8 = cpool.tile_from(a0inv_d)
                    nident = cpool.tile_from(nident_d)
                    hsup4 = cpool.tile_from(hsup_d)
                    stats = cpool.tile([P, 4 * NCH], f32, tag="stats")

            # per-chunk state tiles
            state = {}

            def q_half(sc, half):
                """q tiles for pipe-block pairs (2 per half): matmuls + abs
                drain + mitchell affine + hl8 fp8 conversion."""
                s0 = sc * CH
                if half == 0:
                    state[sc] = {
                        "qabs": wpool.tile(
                            [P, 8, CH], bf16, tag="qabs", bufs=2,
                            name=f"qabs{sc}"),
                        "e852": wpool.tile(
                            [P, 8, CH], i16, tag="e852", bufs=2,
                            name=f"e852{sc}"),
                        "hl8": wpool.tile(
                            [P, 8, CH], f8, tag="hl8", bufs=2,
                            name=f"hl8{sc}"),
                        "rl": wpool.tile(
                            [P, 4, CH], bf16, tag="rl", bufs=2,
                            name=f"rl{sc}"),
                        "sq": wpool.tile(
                            [P, 4, CH], bf16, tag="sq", bufs=2,
                            name=f"sq{sc}"),
                        "dl": wpool.tile(
                            [P, 4, CH], bf16, tag="dl", bufs=2,
                            name=f"dl{sc}"),
                    }
                st = state[sc]
                for h in (2 * half, 2 * half + 1):
                    qp2 = qpool.tile([P, 2, CH], f32, tag="qp", name=f"qp{sc}_{h}")
                    for j in range(2):
                        pc = 2 * h + j
                        nc.tensor.matmul(
                            qp2[:, j, :],
                            invev8[:, :, pc * P : (pc + 1) * P],
                            dt8[:, :, s0 : s0 + CH],
                            start=True, stop=False, perf_mode=DR,
                        )
                        nc.tensor.matmul(
                            qp2[:, j, :], ident16, g8s[sc][:, pc, :],
                            start=False, stop=True,
                        )
                    sl = st["qabs"][:, 2 * h : 2 * h + 2, :]
                    nc.scalar.activation(sl, qp2, AF.Abs)
                    esl = st["e852"][:, 2 * h : 2 * h + 2, :]
                    aeng = nc.vector if h % 2 == 0 else nc.gpsimd
                    aeng.tensor_scalar(
                        esl, sl.bitcast(i16), 870.6, 0.852, OP.add, OP.mult,
                    )
                    nc.vector.tensor_tensor(
                        st["hl8"][:, 2 * h : 2 * h + 2, :],
                        qp2, esl.bitcast(bf16), OP.mult,
                    )

            def h_half(sc, half):
                """H for node-blocks (2 per half): DR matmuls + DVE relu +
                ACT sqrt + Pool dl."""
                st = state[sc]
                for n in (2 * half, 2 * half + 1):
                    hp = hpool.tile([P, CH], f32, tag="hp", name=f"hp{sc}_{n}")
                    for kp in range(4):
                        nc.tensor.matmul(
                            hp,
                            invpt8[:, 2 * kp : 2 * kp + 2, n * P : (n + 1) * P],
                            st["hl8"][:, 2 * kp : 2 * kp + 2, :],
                            start=(kp == 0), stop=(kp == 3), perf_mode=DR,
                        )
                    nc.vector.tensor_scalar(
                        st["rl"][:, n, :], hp, hsup4[:, n : n + 1], 0.0,
                        OP.add, OP.max,
                    )
                eng = nc.gpsimd if sc < NCH - 1 else nc.vector
                for n in (2 * half, 2 * half + 1):
                    nc.scalar.activation(
                        st["sq"][:, n, :], st["rl"][:, n, :], AF.Sqrt,
                    )
                    eng.tensor_tensor(
                        st["dl"][:, n, :], gms[sc][:, n, :],
                        st["sq"][:, n, :], OP.mult,
                    )

            def r_half(sc, half):
                """residual for node-blocks (2 per half): DR + AM inject +
                (-I)*dl inject, ACT Square+accum from PSUM."""
                s0 = sc * CH
                st = state[sc]
                scr = wpool.tile([P, 2, CH], bf16, tag="scr", bufs=2,
                                 name=f"scr{sc}_{half}")
                rps = []
                for n in (2 * half, 2 * half + 1):
                    rp = rpool.tile([P, CH], f32, tag="rp", name=f"rp{sc}_{n}")
                    nc.tensor.matmul(
                        rp,
                        a0inv8[:, :, n * P : (n + 1) * P],
                        dt8[:, :, s0 : s0 + CH],
                        start=True, stop=False, perf_mode=DR,
                    )
                    nc.tensor.matmul(
                        rp, ident16, g8s[sc][:, 8 + n, :],
                        start=False, stop=False,
                    )
                    rps.append(rp)
                for j, n in enumerate((2 * half, 2 * half + 1)):
                    nc.tensor.matmul(
                        rps[j], nident, st["dl"][:, n, :],
                        start=False, stop=True,
                    )
                    col = 4 * sc + 2 * half + j
                    nc.scalar.activation(
                        scr[:, j, :], rps[j], AF.Square,
                        accum_out=stats[:, col : col + 1],
                    )

            # interleaved emission: PE queue alternates q / H / res work so
            # it never drains a whole phase while DVE/ACT catch up
            q_half(0, 0)
            q_half(0, 1)
            q_half(1, 0)
            q_half(1, 1)
            for sc in range(NCH):
                h_half(sc, 0)
                if sc + 2 < NCH:
                    q_half(sc + 2, 0)
                h_half(sc, 1)
                if sc + 2 < NCH:
                    q_half(sc + 2, 1)
                r_half(sc, 0)
                r_half(sc, 1)
            nc.sync.dma_start(out_d, stats)

    nc.compile()
    return nc


def _host_prep(inputs):
    D = np.ascontiguousarray(np.asarray(inputs["D"], np.float32))
    leak = np.asarray(inputs["leak_id"]).reshape(-1).astype(np.int64)
    A0 = np.asarray(inputs["A0"], np.float32).astype(np.float64)
    inv = np.asarray(inputs["inv"], np.float32).astype(np.float64)
    M = np.asarray(inputs["M"], np.float32).astype(np.float64)
    supply = np.asarray(inputs["supply"], np.float32).astype(np.float64)
    L = np.asarray(inputs["L"], np.float32).astype(np.float64)
    d = np.asarray(inputs["d"], np.float32).astype(np.float64)
    C = np.asarray(inputs["C"], np.float32).astype(np.float64)
    a = float(np.asarray(inputs["a"]))
    Cd = float(np.asarray(inputs["Cd"]))
    W1 = np.asarray(inputs["W1"], np.float64)
    b1 = np.asarray(inputs["b1"], np.float64)
    W2 = np.asarray(inputs["W2"], np.float64)
    b2 = np.asarray(inputs["b2"], np.float64)
    W3 = np.asarray(inputs["W3"], np.float64)
    b3 = np.asarray(inputs["b3"], np.float64)
    base = np.asarray(inputs["base"], np.float64)

    ids = np.arange(N_PIPES, dtype=np.float64)[:, None]
    h = np.tanh(ids @ W1 + b1)
    h = np.tanh(h @ W2 + b2)
    table = base + (h @ W3 + b3)[:, 0]

    perm = np.concatenate([np.arange(0, N_NODES, 2), np.arange(1, N_NODES, 2)])
    invp = inv[perm]
    A0p = A0[perm]
    Mp = M[perm]
    inv_ev = invp[:N_DEM]

    K = 10.667 * C**-1.852 * d**-4.871 * L
    k1 = K ** (1.0 / 1.852)
    c0 = Cd * a * math.sqrt(2.0 * G_ACC)

    PM = inv.T @ M
    PMn = (PM * table[None, :]) * k1[:, None]
    AMn = (A0p @ PM) * table[None, :]
    A0inv = A0p @ inv_ev.T
    Ipad = np.zeros((N_NODES, N_DEM))
    Ipad[np.arange(N_DEM), np.arange(N_DEM)] = 1.0

    pm8_t = (PMn / 16.0).T.astype(np.float32).astype(E4)
    am8_t = (AMn * 8.0).T.astype(np.float32).astype(E4)
    m16_t = (Mp * (c0 * 8192.0)).T.astype(np.float32).astype(BF16)

    def blocks3(mat, nb):
        rows, cols = mat.shape
        assert rows == nb * P
        return np.ascontiguousarray(
            np.transpose(mat.reshape(nb, P, cols), (1, 0, 2))
        )

    invev8 = blocks3(
        (inv_ev * k1[None, :] / 128.0).astype(np.float32).astype(E4), 2)
    a0inv8 = blocks3(
        np.ascontiguousarray((A0inv - Ipad).T).astype(np.float32).astype(E4), 2)
    invpt8 = blocks3(
        np.ascontiguousarray((-2.0 * invp).T).astype(np.float32).astype(E4), 8)
    ident16 = (np.eye(P) * 16.0).astype(np.float32).astype(E4)
    nident = (-np.eye(P)).astype(np.float32).astype(BF16)
    hsup4 = np.ascontiguousarray(
        ((invp @ supply) / 4096.0).reshape(4, P).T).astype(np.float32)

    per_core = []
    for c in range(N_CORES):
        Dc = D[c * SC : (c + 1) * SC]
        DT = (Dc.T * 128.0).astype(np.float32).astype(E4)
        dt8 = blocks3(DT, 2)
        lc = leak[c * SC : (c + 1) * SC]
        g8s, gms = [], []
        for sc in range(NCH):
            li = lc[sc * CH : (sc + 1) * CH]
            gpm = blocks3(np.ascontiguousarray(pm8_t[li].T), 8)
            gam = blocks3(np.ascontiguousarray(am8_t[li].T), 4)
            g8s.append(np.ascontiguousarray(
                np.concatenate([gpm, gam], axis=1)))
            gms.append(blocks3(np.ascontiguousarray(m16_t[li].T), 4))
        per_core.append((dt8, g8s, gms))

    shared = {
        "invev8": invev8,
        "a0inv8": a0inv8,
        "invpt8": invpt8,
        "ident16": ident16,
        "nident": nident,
        "hsup4": hsup4,
    }
    return shared, per_core


LAST_RESULTS = None


def kernel(**inputs) -> np.ndarray:
    global LAST_RESULTS
    from concourse.bass_utils import run_bass_kernel_spmd

    shared, per_core = _host_prep(inputs)

    if "nc" not in _MODULE_CACHE:
        _MODULE_CACHE["nc"] = _build_module()
    nc = _MODULE_CACHE["nc"]

    in_maps = []
    for c in range(N_CORES):
        dt8, g8s, gms = per_core[c]
        m = dict(shared)
        m["dt8"] = dt8
        for sc in range(NCH):
            m[f"g8_{sc}"] = g8s[sc]
            m[f"gm_{sc}"] = gms[sc]
        in_maps.append(m)

    import os

    res = run_bass_kernel_spmd(
        nc,
        in_maps,
        core_ids=list(range(N_CORES)),
        trace=bool(os.environ.get("BASS_TRACE")),
    )
    LAST_RESULTS = res

    total = 0.0
    for r in res.results:
        total += float(r["out_stats"].astype(np.float64).sum())
    return np.float32(total / (128.0 * 128.0 * S_TOTAL * N_NODES))


# revision 5
# speedup vs baseline: 1.0275x; 1.0017x over previous
"""Trainium2 Bass kernel v3 for the water-network leak MSE model.

v2 -> v3: interleaved emission (PE never waits a whole phase), 2-bank q PSUM
tiles with ACT Abs drains, Mitchell affine straight off the abs bits (no mask
pass), hl8 TT reads PSUM f32 (sign source, one less drain), relu moved to DVE
tensor_scalar (H weights carry -2), d_leak injected into the residual PSUM by
the PE so ACT Square+accum reads PSUM directly (no DVE rfin pass).

Scales: D*128 fp8, q-weights invev*k1/128, PM table /16 with 16I injects,
hl8 = hL/8192 (Mitchell const (t+870.6)*0.852), H weights -2*invp,
rl = max(hp + hsup/4096, 0) = Hp/4096, sq = sqrt(Hp)/64, M table *c0*8192,
AM table *8 with 16I injects, res weights (A0inv - I), residual = 128*r,
stats accumulate (128r)^2; host divides by 128^2*S*N.
"""

import math

import numpy as np
import ml_dtypes

P = 128
N_CORES = 8
S_TOTAL = 16384
SC = S_TOTAL // N_CORES
CH = 512
NCH = SC // CH
N_NODES = 512
N_PIPES = 1024
N_DEM = 256
G_ACC = 9.80665

BF16 = ml_dtypes.bfloat16
E4 = ml_dtypes.float8_e4m3

_MODULE_CACHE: dict = {}


def _build_module():
    import concourse.bacc as bacc
    import concourse.mybir as mybir
    import concourse.tile as tile

    f32 = mybir.dt.float32
    bf16 = mybir.dt.bfloat16
    f8 = mybir.dt.float8e4
    i16 = mybir.dt.int16
    AF = mybir.ActivationFunctionType
    OP = mybir.AluOpType
    DR = mybir.MatmulPerfMode.DoubleRow

    nc = bacc.Bacc(trn_type="TRN2", target_bir_lowering=False, debug=False)

    import types as _types
    from concourse.hw_specs import get_activation_tables as _gat
    import bass_rust as _bass_rust

    _OURS = {AF.Abs, AF.Relu, AF.Square, AF.Sqrt, AF.Identity, AF.Copy,
             AF.Sign, AF.MemsetZero}

    def _patched_act_table_loads(self):
        has_activation = any(
            isinstance(i, mybir.InstActivation)
            for b in self.main_func.blocks
            for i in b.instructions
        )
        if not has_activation:
            return
        tables = []
        for name, fns in _gat(self.m.arch).items():
            if name != "sqrt_and_others":
                fns = fns - _OURS
            tables.append((name, fns))
        _bass_rust.insert_act_table_loads(self, tables)

    nc.insert_act_table_loads = _types.MethodType(_patched_act_table_loads, nc)

    # m1 = [ident16 | invev8 | dt8], m2 = [invpt8 | a0inv8]: one DMA each
    m1_d = nc.dram_tensor("m1", [P, P + 2 * N_PIPES + 2 * SC], f8,
                          kind="ExternalInput").ap()
    m2_d = nc.dram_tensor("m2", [P, 8 * N_NODES + 2 * N_NODES], f8,
                          kind="ExternalInput").ap()
    nident_d = nc.dram_tensor("nident", [P, P], bf16, kind="ExternalInput").ap()
    hsup_d = nc.dram_tensor("hsup4", [P, 4], f32, kind="ExternalInput").ap()
    g8_ds = [
        nc.dram_tensor(f"g8_{c}", [P, 12, CH], f8, kind="ExternalInput").ap()
        for c in range(NCH)
    ]
    gm_ds = [
        nc.dram_tensor(f"gm_{c}", [P, 4, CH], bf16, kind="ExternalInput").ap()
        for c in range(NCH)
    ]
    out_d = nc.dram_tensor("out_stats", [P, 4 * NCH], f32, kind="ExternalOutput").ap()

    with tile.TileContext(nc) as tc:
        with (
            tc.tile_pool(name="const", bufs=1) as cpool,
            tc.tile_pool(name="gat", bufs=1) as gpool,
            tc.tile_pool(name="work", bufs=1) as wpool,
            tc.tile_pool(name="qps", bufs=2, space="PSUM") as qpool,
            tc.tile_pool(name="hps", bufs=2, space="PSUM") as hpool,
            tc.tile_pool(name="rps", bufs=2, space="PSUM") as rpool,
        ):
            m1t = cpool.tile_from(m1_d)
            ident16 = m1t[:, 0:P]
            invev8 = m1t[:, P : P + 2 * N_PIPES].rearrange(
                "p (k c) -> p k c", k=2)
            dt8 = m1t[:, P + 2 * N_PIPES :].rearrange(
                "p (k c) -> p k c", k=2)

            g8s, gms = [], []
            for sc in range(NCH):
                g8 = gpool.tile([P, 12, CH], f8, tag="g8", bufs=3, name=f"g8t{sc}")
                gm = gpool.tile([P, 4, CH], bf16, tag="gm", bufs=3, name=f"gmt{sc}")
                nc.sync.dma_start(g8, g8_ds[sc])
                nc.sync.dma_start(gm, gm_ds[sc])
                g8s.append(g8)
                gms.append(gm)
                if sc == 0:
                    m2t = cpool.tile_from(m2_d)
                    invpt8 = m2t[:, 0 : 8 * N_NODES].rearrange(
                        "p (k c) -> p k c", k=8)
                    a0inv8 = m2t[:, 8 * N_NODES :].rearrange(
                        "p (k c) -> p k c", k=2)
                    nident = cpool.tile_from(nident_d)
                    hsup4 = cpool.tile_from(hsup_d)
                    stats = cpool.tile([P, 4 * NCH], f32, tag="stats")

            # per-chunk state tiles
            state = {}

            def q_half(sc, half):
                """q tiles for pipe-block pairs (2 per half): matmuls + abs
                drain + mitchell affine + hl8 fp8 conversion."""
                s0 = sc * CH
                if half == 0:
                    state[sc] = {
                        "qabs": wpool.tile(
                            [P, 8, CH], bf16, tag="qabs", bufs=2,
                            name=f"qabs{sc}"),
                        "e852": wpool.tile(
                            [P, 8, CH], i16, tag="e852", bufs=2,
                            name=f"e852{sc}"),
                        "hl8": wpool.tile(
                            [P, 8, CH], f8, tag="hl8", bufs=2,
                            name=f"hl8{sc}"),
                        "rl": wpool.tile(
                            [P, 4, CH], bf16, tag="rl", bufs=2,
                            name=f"rl{sc}"),
                        "sq": wpool.tile(
                            [P, 4, CH], bf16, tag="sq", bufs=2,
                            name=f"sq{sc}"),
                        "dl": wpool.tile(
                            [P, 4, CH], bf16, tag="dl", bufs=2,
                            name=f"dl{sc}"),
                    }
                st = state[sc]
                for h in (2 * half, 2 * half + 1):
                    qp2 = qpool.tile([P, 2, CH], f32, tag="qp", name=f"qp{sc}_{h}")
                    for j in range(2):
                        pc = 2 * h + j
                        nc.tensor.matmul(
                            qp2[:, j, :],
                            invev8[:, :, pc * P : (pc + 1) * P],
                            dt8[:, :, s0 : s0 + CH],
                            start=True, stop=False, perf_mode=DR,
                        )
                        nc.tensor.matmul(
                            qp2[:, j, :], ident16, g8s[sc][:, pc, :],
                            start=False, stop=True,
                        )
                    sl = st["qabs"][:, 2 * h : 2 * h + 2, :]
                    nc.scalar.activation(sl, qp2, AF.Abs)
                    esl = st["e852"][:, 2 * h : 2 * h + 2, :]
                    aeng = nc.vector if h % 2 == 0 else nc.gpsimd
                    aeng.tensor_scalar(
                        esl, sl.bitcast(i16), 870.6, 0.852, OP.add, OP.mult,
                    )
                    nc.vector.tensor_tensor(
                        st["hl8"][:, 2 * h : 2 * h + 2, :],
                        qp2, esl.bitcast(bf16), OP.mult,
                    )

            def h_half(sc, half):
                """H for node-blocks (2 per half): DR matmuls + DVE relu +
                ACT sqrt + Pool dl."""
                st = state[sc]
                for n in (2 * half, 2 * half + 1):
                    hp = hpool.tile([P, CH], f32, tag="hp", name=f"hp{sc}_{n}")
                    for kp in range(4):
                        nc.tensor.matmul(
                            hp,
                            invpt8[:, 2 * kp : 2 * kp + 2, n * P : (n + 1) * P],
                            st["hl8"][:, 2 * kp : 2 * kp + 2, :],
                            start=(kp == 0), stop=(kp == 3), perf_mode=DR,
                        )
                    nc.vector.tensor_scalar(
                        st["rl"][:, n, :], hp, hsup4[:, n : n + 1], 0.0,
                        OP.add, OP.max,
                    )
                eng = nc.gpsimd if sc < NCH - 1 else nc.vector
                for n in (2 * half, 2 * half + 1):
                    nc.scalar.activation(
                        st["sq"][:, n, :], st["rl"][:, n, :], AF.Sqrt,
                    )
                    eng.tensor_tensor(
                        st["dl"][:, n, :], gms[sc][:, n, :],
                        st["sq"][:, n, :], OP.mult,
                    )

            def r_half(sc, half):
                """residual for node-blocks (2 per half): DR + AM inject +
                (-I)*dl inject, ACT Square+accum from PSUM."""
                s0 = sc * CH
                st = state[sc]
                scr = wpool.tile([P, 2, CH], bf16, tag="scr", bufs=2,
                                 name=f"scr{sc}_{half}")
                rps = []
                for n in (2 * half, 2 * half + 1):
                    rp = rpool.tile([P, CH], f32, tag="rp", name=f"rp{sc}_{n}")
                    nc.tensor.matmul(
                        rp,
                        a0inv8[:, :, n * P : (n + 1) * P],
                        dt8[:, :, s0 : s0 + CH],
                        start=True, stop=False, perf_mode=DR,
                    )
                    nc.tensor.matmul(
                        rp, ident16, g8s[sc][:, 8 + n, :],
                        start=False, stop=False,
                    )
                    rps.append(rp)
                for j, n in enumerate((2 * half, 2 * half + 1)):
                    nc.tensor.matmul(
                        rps[j], nident, st["dl"][:, n, :],
                        start=False, stop=True,
                    )
                    col = 4 * sc + 2 * half + j
                    nc.scalar.activation(
                        scr[:, j, :], rps[j], AF.Square,
                        accum_out=stats[:, col : col + 1],
                    )

            # interleaved emission: PE queue alternates q / H / res work so
            # it never drains a whole phase while DVE/ACT catch up
            q_half(0, 0)
            q_half(0, 1)
            q_half(1, 0)
            q_half(1, 1)
            for sc in range(NCH):
                h_half(sc, 0)
                if sc + 2 < NCH:
                    q_half(sc + 2, 0)
                h_half(sc, 1)
                if sc + 2 < NCH:
                    q_half(sc + 2, 1)
                r_half(sc, 0)
                r_half(sc, 1)
            nc.sync.dma_start(out_d, stats)

    nc.compile()
    return nc


def _host_prep(inputs):
    D = np.ascontiguousarray(np.asarray(inputs["D"], np.float32))
    leak = np.asarray(inputs["leak_id"]).reshape(-1).astype(np.int64)
    A0 = np.asarray(inputs["A0"], np.float32).astype(np.float64)
    inv = np.asarray(inputs["inv"], np.float32).astype(np.float64)
    M = np.asarray(inputs["M"], np.float32).astype(np.float64)
    supply = np.asarray(inputs["supply"], np.float32).astype(np.float64)
    L = np.asarray(inputs["L"], np.float32).astype(np.float64)
    d = np.asarray(inputs["d"], np.float32).astype(np.float64)
    C = np.asarray(inputs["C"], np.float32).astype(np.float64)
    a = float(np.asarray(inputs["a"]))
    Cd = float(np.asarray(inputs["Cd"]))
    W1 = np.asarray(inputs["W1"], np.float64)
    b1 = np.asarray(inputs["b1"], np.float64)
    W2 = np.asarray(inputs["W2"], np.float64)
    b2 = np.asarray(inputs["b2"], np.float64)
    W3 = np.asarray(inputs["W3"], np.float64)
    b3 = np.asarray(inputs["b3"], np.float64)
    base = np.asarray(inputs["base"], np.float64)

    ids = np.arange(N_PIPES, dtype=np.float64)[:, None]
    h = np.tanh(ids @ W1 + b1)
    h = np.tanh(h @ W2 + b2)
    table = base + (h @ W3 + b3)[:, 0]

    perm = np.concatenate([np.arange(0, N_NODES, 2), np.arange(1, N_NODES, 2)])
    invp = inv[perm]
    A0p = A0[perm]
    Mp = M[perm]
    inv_ev = invp[:N_DEM]

    K = 10.667 * C**-1.852 * d**-4.871 * L
    k1 = K ** (1.0 / 1.852)
    c0 = Cd * a * math.sqrt(2.0 * G_ACC)

    PM = inv.T @ M
    PMn = (PM * table[None, :]) * k1[:, None]
    AMn = (A0p @ PM) * table[None, :]
    A0inv = A0p @ inv_ev.T
    Ipad = np.zeros((N_NODES, N_DEM))
    Ipad[np.arange(N_DEM), np.arange(N_DEM)] = 1.0

    pm8_t = (PMn / 16.0).T.astype(np.float32).astype(E4)
    am8_t = (AMn * 8.0).T.astype(np.float32).astype(E4)
    m16_t = (Mp * (c0 * 8192.0)).T.astype(np.float32).astype(BF16)

    def blocks3(mat, nb):
        rows, cols = mat.shape
        assert rows == nb * P
        return np.ascontiguousarray(
            np.transpose(mat.reshape(nb, P, cols), (1, 0, 2))
        )

    invev8 = blocks3(
        (inv_ev * k1[None, :] / 128.0).astype(np.float32).astype(E4), 2)
    a0inv8 = blocks3(
        np.ascontiguousarray((A0inv - Ipad).T).astype(np.float32).astype(E4), 2)
    invpt8 = blocks3(
        np.ascontiguousarray((-2.0 * invp).T).astype(np.float32).astype(E4), 8)
    ident16 = (np.eye(P) * 16.0).astype(np.float32).astype(E4)
    nident = (-np.eye(P)).astype(np.float32).astype(BF16)
    hsup4 = np.ascontiguousarray(
        ((invp @ supply) / 4096.0).reshape(4, P).T).astype(np.float32)

    per_core = []
    for c in range(N_CORES):
        Dc = D[c * SC : (c + 1) * SC]
        DT = (Dc.T * 128.0).astype(np.float32).astype(E4)
        dt8 = blocks3(DT, 2)
        lc = leak[c * SC : (c + 1) * SC]
        g8s, gms = [], []
        for sc in range(NCH):
            li = lc[sc * CH : (sc + 1) * CH]
            gpm = blocks3(np.ascontiguousarray(pm8_t[li].T), 8)
            gam = blocks3(np.ascontiguousarray(am8_t[li].T), 4)
            g8s.append(np.ascontiguousarray(
                np.concatenate([gpm, gam], axis=1)))
            gms.append(blocks3(np.ascontiguousarray(m16_t[li].T), 4))
        per_core.append((dt8, g8s, gms))

    m2 = np.concatenate(
        [invpt8.reshape(P, -1), a0inv8.reshape(P, -1)], axis=1)
    shared = {
        "m2": np.ascontiguousarray(m2),
        "nident": nident,
        "hsup4": hsup4,
    }
    return shared, per_core, ident16, invev8


LAST_RESULTS = None


def kernel(**inputs) -> np.ndarray:
    global LAST_RESULTS
    from concourse.bass_utils import run_bass_kernel_spmd

    shared, per_core, ident16, invev8 = _host_prep(inputs)

    if "nc" not in _MODULE_CACHE:
        _MODULE_CACHE["nc"] = _build_module()
    nc = _MODULE_CACHE["nc"]

    in_maps = []
    for c in range(N_CORES):
        dt8, g8s, gms = per_core[c]
        m = dict(shared)
        m["m1"] = np.ascontiguousarray(np.concatenate(
            [ident16, invev8.reshape(P, -1), dt8.reshape(P, -1)], axis=1))
        for sc in range(NCH):
            m[f"g8_{sc}"] = g8s[sc]
            m[f"gm_{sc}"] = gms[sc]
        in_maps.append(m)

    import os

    res = run_bass_kernel_spmd(
        nc,
        in_maps,
        core_ids=list(range(N_CORES)),
        trace=bool(os.environ.get("BASS_TRACE")),
    )
    LAST_RESULTS = res

    total = 0.0
    for r in res.results:
        total += float(r["out_stats"].astype(np.float64).sum())
    return np.float32(total / (128.0 * 128.0 * S_TOTAL * N_NODES))
